# revision 16
# baseline (speedup 1.0000x reference)
"""AttnGCN (2-layer GATv2 + BN + FC) Trainium2 kernel, 8-core SPMD.

Target-node sharding: edges (plus self-loops) are sorted by target and
bucketed into 128-node target blocks; each core owns a contiguous block range.
Softmax is computed in one pass via out = (sum_e exp(a_e) x_src) / sum_e
exp(a_e) (the max-shift of the reference is a mathematical no-op).  Messages
aggregate the *raw* gathered source rows with exp-scaled one-hot matmuls; the
Wl projection is applied once per node block after aggregation (associativity),
in transposed orientation so the BatchNorm affine is a per-partition ACT op.
Layer-1 node outputs are exchanged with an AllGather between layers.
"""

import sys

import numpy as np

sys.path.insert(0, "/opt/trn_rl_repo")

N = 100000
H, C = 2, 128
IN, HID, OUT = 5, 128, 5
HC = H * C

BLK = 128      # target nodes per block
GRP = 128      # edges per group
GSG = 8        # groups per gather-supergroup
PAD_TGT = 200.0
SPLIT_GATHER = True


def _host_prep(h, edge_index, edge_weight, n, cores):
    nb_total = -(-((n + BLK - 1) // BLK) // cores) * cores
    bpc = nb_total // cores
    npad = nb_total * BLK

    src = edge_index[0].astype(np.int64)
    tgt = edge_index[1].astype(np.int64)
    ew = edge_weight[:, 0].astype(np.float32)
    ew_mean = np.float32(edge_weight.astype(np.float32).mean())

    loops = np.arange(npad, dtype=np.int64)
    src = np.concatenate([src, loops])
    tgt = np.concatenate([tgt, loops])
    ew = np.concatenate(
        [ew, np.full(n, ew_mean, np.float32), np.zeros(npad - n, np.float32)]
    )

    order = np.argsort(tgt, kind="stable")
    src, tgt, ew = src[order], tgt[order], ew[order]

    counts = np.bincount(tgt // BLK, minlength=nb_total)
    gpb = int((counts.max() + GRP - 1) // GRP)
    while (bpc * gpb) % GSG != 0:
        gpb += 1
    gpc = bpc * gpb
    nsg = gpc // GSG

    blk_starts = np.zeros(nb_total + 1, np.int64)
    np.cumsum(counts, out=blk_starts[1:])

    nodedeg = np.maximum(np.bincount(tgt, minlength=npad), 1).astype(np.float32)
    invd_all = (1.0 / nodedeg).astype(np.float32)

    per_core = []
    hpad = np.zeros((npad, IN), np.float32)
    hpad[:n] = h.astype(np.float32)
    for c in range(cores):
        IDX = np.zeros((gpc, GRP), np.int32)
        TGL = np.full((gpc, GRP), PAD_TGT, np.float32)
        EWG = np.zeros((gpc, GRP), np.float32)
        g0 = 0
        for b in range(c * bpc, (c + 1) * bpc):
            s, e = int(blk_starts[b]), int(blk_starts[b + 1])
            m = e - s
            fi = np.arange(g0 * GRP, g0 * GRP + m)
            IDX.reshape(-1)[fi] = src[s:e]
            TGL.reshape(-1)[fi] = (tgt[s:e] - b * BLK).astype(np.float32)
            EWG.reshape(-1)[fi] = ew[s:e]
            g0 += gpb
        assert g0 == gpc

        idxt = np.zeros((nsg, GRP, GSG), np.int32)
        tgtc = np.zeros((nsg, GRP, GSG), np.float32)
        tgtr = np.zeros((nsg, 1, GSG * GRP), np.float32)
        ewt = np.zeros((nsg, 2, GSG * GRP), np.float32)
        for sg in range(nsg):
            for j in range(GSG):
                g = sg * GSG + j
                idxt[sg, :, j] = IDX[g]
                tgtc[sg, :, j] = TGL[g]
                tgtr[sg, 0, j * GRP:(j + 1) * GRP] = TGL[g]
                ewt[sg, 0, j * GRP:(j + 1) * GRP] = EWG[g]
        ewt[:, 1, :] = 1.0

        invd = invd_all[c * bpc * BLK:(c + 1) * bpc * BLK]
        per_core.append(dict(
            IDXT=idxt, TGTC=tgtc, TGTR=tgtr, EWT=ewt,
            INVC=np.ascontiguousarray(invd.reshape(bpc, BLK, 1)),
            INVR=np.ascontiguousarray(invd.reshape(bpc, 1, BLK)),
            HTC=np.ascontiguousarray(
                hpad[c * bpc * BLK:(c + 1) * bpc * BLK].T),
        ))
    return per_core, hpad, gpb, bpc, nsg, npad


def _weights_host(p):
    eps = np.float32(1e-5)

    def affine(g, b, m, v, bias):
        s = (np.asarray(g) / np.sqrt(np.asarray(v) + eps)).astype(np.float32)
        bb = (np.asarray(b) + (np.asarray(bias) - np.asarray(m)) * s).astype(np.float32)
        return s, bb

    s1, b1 = affine(p["bn1g"], p["bn1b"], p["bn1m"], p["bn1v"], p["bias1"])
    s2, b2 = affine(p["bn2g"], p["bn2b"], p["bn2m"], p["bn2v"], p["bias2"])
    att1 = np.asarray(p["att1"], np.float32).reshape(-1)
    att2 = np.asarray(p["att2"], np.float32).reshape(-1)
    bl1 = np.asarray(p["bl1"], np.float32)
    bl2 = np.asarray(p["bl2"], np.float32)
    return dict(
        WL1=np.asarray(p["Wl1"], np.float32),
        WR1=np.asarray(p["Wr1"], np.float32),
        WB1=np.stack([np.asarray(p["We1"], np.float32)[0],
                      np.asarray(p["br1"], np.float32)
                      + np.asarray(p["bl1"], np.float32)]),
        WL2=np.asarray(p["Wl2"], np.float32),
        WR2=np.asarray(p["Wr2"], np.float32),
        WB2=np.stack([np.asarray(p["We2"], np.float32)[0],
                      np.asarray(p["br2"], np.float32)
                      + np.asarray(p["bl2"], np.float32)]),
        ATT1=np.tile(att1[None, :], (128, 1)),
        ATT2=np.tile(att2[None, :], (128, 1)),
        SM1=s1[:, None], BM1=b1[:, None], SM2=s2[:, None], BM2=b2[:, None],
        BL1=(0.5 * (bl1[:C] + bl1[C:]))[None, :],
        BL2=(0.5 * (bl2[:C] + bl2[C:]))[None, :],
        FCW=np.asarray(p["fcw"], np.float32),
        FCB=np.asarray(p["fcb"], np.float32)[:, None],
    )


def _build_nc(gpb, bpc, nsg, npad, cores, f32r=True):
    import concourse.bass as bass
    import concourse.bacc as bacc
    import concourse.mybir as mybir
    import concourse.tile as tile
    from concourse.masks import make_identity
    from contextlib import ExitStack

    dt = mybir.dt
    AF = mybir.ActivationFunctionType
    ALU = mybir.AluOpType
    nsl = bpc * BLK

    nc = bacc.Bacc()

    dtr = dt.float32r if f32r else dt.float32

    def inp(name, shape, dtype=dt.float32):
        if dtype is None:
            dtype = dtr
        return nc.dram_tensor(name, shape, dtype, kind="ExternalInput")

    IDXT = inp("IDXT", [nsg, GRP, GSG], dt.int32)
    TGTC = inp("TGTC", [nsg, GRP, GSG])
    TGTR = inp("TGTR", [nsg, 1, GSG * GRP], None)
    EWT = inp("EWT", [nsg, 2, GSG * GRP], None)
    INVC = inp("INVC", [bpc, BLK, 1])
    INVR = inp("INVR", [bpc, 1, BLK], None)
    H5 = inp("H5", [npad, IN], None)
    HTC = inp("HTC", [IN, nsl], None)
    RCONST = {"WL1", "WR1", "WB1", "WL2", "WR2", "WB2", "BL1", "BL2"}
    CONSTS = {nm: inp(nm, sh, None if nm in RCONST else dt.float32)
              for nm, sh in [
        ("WL1", [IN, HC]), ("WR1", [IN, HC]), ("WB1", [2, HC]),
        ("WL2", [HID, HC]), ("WR2", [HID, HC]), ("WB2", [2, HC]),
        ("ATT1", [128, HC]), ("ATT2", [128, HC]),
        ("SM1", [C, 1]), ("BM1", [C, 1]), ("SM2", [C, 1]), ("BM2", [C, 1]),
        ("BL1", [1, C]), ("BL2", [1, C]),
        ("FCW", [HID, OUT]), ("FCB", [OUT, 1]),
    ]}
    OUTT = nc.dram_tensor("OUTT", [OUT, nsl], dt.float32, kind="ExternalOutput")

    def mmdt(ap):
        return ap

    with ExitStack() as ctx:
        tc = ctx.enter_context(tile.TileContext(nc))
        cpool = ctx.enter_context(tc.tile_pool(name="consts", bufs=1))
        spool = ctx.enter_context(tc.tile_pool(name="sg", bufs=2))
        gpool = ctx.enter_context(tc.tile_pool(name="grp", bufs=4))
        bpool = ctx.enter_context(tc.tile_pool(name="blk", bufs=3))
        dpool = ctx.enter_context(tc.tile_pool(name="dram", bufs=1, space="DRAM"))
        pt = ctx.enter_context(tc.tile_pool(name="pt", bufs=1, space="PSUM"))
        ps = ctx.enter_context(tc.tile_pool(name="ps", bufs=2, space="PSUM"))
        px = ctx.enter_context(tc.tile_pool(name="px", bufs=1, space="PSUM"))
        pxt = ctx.enter_context(tc.tile_pool(name="pxt", bufs=2, space="PSUM"))

        ident = cpool.tile([128, 128], dt.float32)
        make_identity(nc, ident[:])
        iota_i = cpool.tile([128, 128], dt.int32)
        nc.gpsimd.iota(iota_i[:], pattern=[[1, 128]], base=0, channel_multiplier=0)
        iota_m = cpool.tile([128, 128], dt.float32)
        nc.vector.tensor_copy(iota_m[:], iota_i[:])
        ones_r = cpool.tile([1, 128], dtr)
        nc.gpsimd.memset(ones_r[:].bitcast(dt.float32), 1.0)
        iotap_i = cpool.tile([128, 1], dt.int32)
        nc.gpsimd.iota(iotap_i[:], pattern=[[1, 1]], base=0, channel_multiplier=1)
        iota_p = cpool.tile([128, 1], dt.float32)
        nc.vector.tensor_copy(iota_p[:], iotap_i[:])

        cs = {}
        for nm, t in CONSTS.items():
            cdt = dtr if nm in RCONST else dt.float32
            til = cpool.tile(list(t.shape), cdt, name=f"c_{nm}")
            nc.sync.dma_start(out=til[:], in_=t[:, :])
            cs[nm] = til

        YS = dpool.tile([nsl, HID], dtr, name="YS")
        YF = dpool.tile([npad, HID], dtr, name="YF")

        def epilogue(li, b, s_t, wl, sm, bm, bl):
            D = IN if li == 1 else HID
            invc = bpool.tile([BLK, 1], dt.float32, tag="invc")
            nc.sync.dma_start(out=invc[:], in_=INVC[b])
            invr = bpool.tile([1, BLK], dtr, tag="invr")
            nc.sync.dma_start(out=invr[:], in_=INVR[b])
            cf = bpool.tile([BLK, 2 * D], dt.float32, tag="cf")
            for hh in range(H):
                rec = bpool.tile([BLK, 1], dt.float32, tag=f"rec{hh}")
                nc.vector.reciprocal(rec[:], s_t[hh][:, D:D + 1])
                f = bpool.tile([BLK, 1], dt.float32, tag=f"f{hh}")
                nc.vector.tensor_scalar(
                    out=f[:], in0=rec[:], scalar1=invc[:], scalar2=0.5,
                    op0=ALU.mult, op1=ALU.mult)
                nc.vector.tensor_scalar(
                    out=cf[:, hh * D:(hh + 1) * D], in0=s_t[hh][:, 0:D],
                    scalar1=f[:], scalar2=None, op0=ALU.mult)
            cft_ps = px.tile([D, 2 * BLK], dt.float32, tag="px")
            for hh in range(H):
                nc.tensor.transpose(out=cft_ps[:, hh * BLK:(hh + 1) * BLK],
                                    in_=cf[:, hh * D:(hh + 1) * D],
                                    identity=ident[:])
            cft = bpool.tile([D, 2 * BLK], dtr, tag="cft")
            nc.vector.tensor_copy(cft[:], cft_ps[:])
            qt_ps = px.tile([C, BLK], dt.float32, tag="px")
            for hh in range(H):
                nc.tensor.matmul(out=qt_ps[:],
                                 lhsT=mmdt(wl[0:D, hh * C:(hh + 1) * C]),
                                 rhs=mmdt(cft[:, hh * BLK:(hh + 1) * BLK]),
                                 start=(hh == 0), stop=False,
                                 skip_group_check=True)
            nc.tensor.matmul(out=qt_ps[:], lhsT=mmdt(bl[:]), rhs=mmdt(invr[:]),
                             start=False, stop=True, skip_group_check=True)
            yt = bpool.tile([C, BLK], dt.float32, tag="yt")
            nc.scalar.activation(yt[:], qt_ps[:], AF.Lrelu,
                                 bias=bm[:], scale=sm[:], alpha=0.01)
            if li == 1:
                ytr_ps = px.tile([BLK, C], dt.float32, tag="px")
                nc.tensor.transpose(out=ytr_ps[:], in_=yt[:], identity=ident[:])
                ysb = bpool.tile([BLK, C], dtr, tag="ysb")
                nc.vector.tensor_copy(ysb[:], ytr_ps[:])
                nc.sync.dma_start(out=YS[b * BLK:(b + 1) * BLK, :], in_=ysb[:])
            else:
                o_ps = px.tile([OUT, BLK], dt.float32, tag="px")
                nc.tensor.matmul(out=o_ps[:], lhsT=mmdt(cs["FCW"][:]),
                                 rhs=mmdt(yt[:]), start=True, stop=True,
                                 skip_group_check=True)
                osb = bpool.tile([OUT, BLK], dt.float32, tag="osb")
                nc.vector.tensor_scalar(out=osb[:], in0=o_ps[:],
                                        scalar1=cs["FCB"][:], scalar2=None,
                                        op0=ALU.add)
                nc.sync.dma_start(out=OUTT[:, b * BLK:(b + 1) * BLK], in_=osb[:])

        def layer(li):
            D = IN if li == 1 else HID
            att = cs["ATT1"] if li == 1 else cs["ATT2"]
            wl = cs["WL1"] if li == 1 else cs["WL2"]
            wr = cs["WR1"] if li == 1 else cs["WR2"]
            wb = cs["WB1"] if li == 1 else cs["WB2"]
            sm = cs["SM1"] if li == 1 else cs["SM2"]
            bm = cs["BM1"] if li == 1 else cs["BM2"]
            bl = cs["BL1"] if li == 1 else cs["BL2"]
            gsrc = H5[:, :] if li == 1 else YF[:, :]

            xrb_of = {}
            s_of = {}
            for sg in range(nsg):
                idx8 = spool.tile([GRP, GSG], dt.int32, tag="idx8")
                nc.sync.dma_start(out=idx8[:], in_=IDXT[sg])
                tgc8 = spool.tile([GRP, GSG], dt.float32, tag="tgc8")
                nc.sync.dma_start(out=tgc8[:], in_=TGTC[sg])
                tgr8 = spool.tile([1, GSG * GRP], dtr, tag="tgr8")
                nc.sync.dma_start(out=tgr8[:], in_=TGTR[sg])
                ew8 = spool.tile([2, GSG * GRP], dtr, tag="ew8")
                nc.sync.dma_start(out=ew8[:], in_=EWT[sg])

                r8 = spool.tile([GRP, GSG, 256], dtr, tag="r8")
                nc.gpsimd.memset(r8[:, :, D:].bitcast(dt.float32), 0.0)
                if SPLIT_GATHER:
                    for jg in range(GSG):
                        nc.gpsimd.indirect_dma_start(
                            out=r8[:, jg, 0:D], out_offset=None, in_=gsrc,
                            in_offset=bass.IndirectOffsetOnAxis(
                                ap=idx8[:, jg:jg + 1], axis=0))
                else:
                    nc.gpsimd.indirect_dma_start(
                        out=r8[:, :, 0:D], out_offset=None, in_=gsrc,
                        in_offset=bass.IndirectOffsetOnAxis(ap=idx8[:, :], axis=0))
                nc.gpsimd.memset(r8[:, :, D:D + 1].bitcast(dt.float32), 1.0)

                ot8 = spool.tile([BLK, GSG * GRP], dtr, tag="ot8")
                for half in range(GSG * GRP // 512):
                    hsl = slice(half * 512, (half + 1) * 512)
                    tgrm = px.tile([BLK, 512], dt.float32, tag="px")
                    nc.tensor.matmul(out=tgrm[:], lhsT=mmdt(ones_r[:]),
                                     rhs=mmdt(tgr8[:, hsl]), start=True,
                                     stop=True, skip_group_check=True)
                    nc.vector.tensor_scalar(
                        out=ot8[:, hsl], in0=tgrm[:],
                        scalar1=iota_p[:], scalar2=None, op0=ALU.is_equal)

                alph8 = spool.tile([GRP, GSG, H], dt.float32, tag="alph8")
                ex8 = spool.tile([GRP, GSG, H], dt.float32, tag="ex8")

                for j in range(GSG):
                    g = sg * GSG + j
                    b, bj = divmod(g, gpb)
                    if bj == 0:
                        # build per-block XR tile (x_block @ Wr, no bias; br
                        # is added via the [ew;1]@[We;br] matmul)
                        if li == 1:
                            xl_ = bpool.tile([IN, BLK], dtr, tag="xrl")
                            nc.sync.dma_start(
                                out=xl_[:], in_=HTC[:, b * BLK:(b + 1) * BLK])
                            lhs_xr = xl_
                        else:
                            ybl = bpool.tile([BLK, HID], dtr, tag="ybl")
                            nc.sync.dma_start(
                                out=ybl[:], in_=YS[b * BLK:(b + 1) * BLK, :])
                            ytp = px.tile([HID, BLK], dt.float32, tag="px")
                            nc.tensor.transpose(out=ytp[:], in_=ybl[:].bitcast(dt.float32),
                                                identity=ident[:])
                            ytb = bpool.tile([HID, BLK], dtr, tag="ytb")
                            nc.vector.tensor_copy(ytb[:], ytp[:])
                            lhs_xr = ytb
                        xr_ps = px.tile([BLK, HC], dt.float32, tag="px")
                        nc.tensor.matmul(out=xr_ps[:], lhsT=mmdt(lhs_xr[:]),
                                         rhs=mmdt(wr[:]), start=True, stop=True,
                                         skip_group_check=True)
                        xrb = bpool.tile([BLK, HC], dtr, tag="xrb")
                        nc.vector.tensor_copy(xrb[:], xr_ps[:])
                        xrb_of[b] = xrb
                    if j % 2 == 0:
                        xt_ps = pxt.tile([D, 2 * GRP], dt.float32, tag="xt")
                        for jj in (0, 1):
                            nc.tensor.transpose(
                                out=xt_ps[:, jj * GRP:(jj + 1) * GRP],
                                in_=r8[:, j + jj, 0:D].bitcast(dt.float32), identity=ident[:])
                        xt2 = gpool.tile([D, 2 * GRP], dtr, tag="xt2")
                        nc.vector.tensor_copy(xt2[:], xt_ps[:])
                    esl = slice(j * GRP, (j + 1) * GRP)
                    e2 = slice((j % 2) * GRP, (j % 2 + 1) * GRP)
                    t_ps = pt.tile([GRP, HC], dt.float32, tag="t")
                    nc.tensor.matmul(out=t_ps[:], lhsT=mmdt(xt2[:, e2]),
                                     rhs=mmdt(wl[:]), start=True, stop=False,
                                     skip_group_check=True)
                    nc.tensor.matmul(out=t_ps[:], lhsT=mmdt(ot8[:, esl]),
                                     rhs=mmdt(xrb_of[b][:]), start=False,
                                     stop=False, skip_group_check=True)
                    nc.tensor.matmul(out=t_ps[:], lhsT=mmdt(ew8[:, esl]),
                                     rhs=mmdt(wb[:]), start=False, stop=True,
                                     skip_group_check=True)
                    zl = gpool.tile([GRP, HC], dt.float32, tag="zl")
                    nc.scalar.activation(zl[:], t_ps[:], AF.Lrelu, alpha=0.2)
                    za = gpool.tile([GRP, H, C], dt.float32, tag="za")
                    nc.vector.tensor_mul(za[:, :, :], zl[:].rearrange(
                        "p (h c) -> p h c", h=H), att[:].rearrange(
                        "p (h c) -> p h c", h=H))
                    nc.vector.tensor_reduce(
                        out=alph8[:, j, :], in_=za[:, :, :],
                        axis=mybir.AxisListType.X, op=ALU.add)

                nc.scalar.activation(ex8[:, :, :], alph8[:, :, :], AF.Exp)

                for j in range(GSG):
                    g = sg * GSG + j
                    b, bj = divmod(g, gpb)
                    if bj == 0:
                        s_of[b] = [ps.tile([BLK, 256], dt.float32, tag=f"s{hh}",
                                           name=f"s{hh}_{li}_{b}")
                                   for hh in range(H)]
                    for hh in range(H):
                        oh = gpool.tile([GRP, BLK], dtr, tag=f"oh{hh}")
                        nc.vector.tensor_scalar(
                            out=oh[:], in0=iota_m[:], scalar1=tgc8[:, j:j + 1],
                            scalar2=ex8[:, j, hh:hh + 1],
                            op0=ALU.is_equal, op1=ALU.mult)
                        nc.tensor.matmul(
                            out=s_of[b][hh][:], lhsT=mmdt(oh[:]),
                            rhs=mmdt(r8[:, j, :]),
                            start=(bj == 0), stop=(bj == gpb - 1),
                            skip_group_check=True)
                    if bj == gpb - 1:
                        epilogue(li, b, s_of.pop(b), wl, sm, bm, bl)
                        xrb_of.pop(b, None)

        layer(1)
        if cores > 1:
            nc.gpsimd.collective_compute(
                "AllGather", mybir.AluOpType.bypass,
                replica_groups=[list(range(cores))],
                ins=[YS[:, :]], outs=[YF[:, :]])
        else:
            nc.sync.dma_start(out=YF[:, :], in_=YS[:, :])
        layer(2)

    nc.compile()
    return nc


def prepare(inputs, n, cores, f32r=True):
    """Build nc + per-core input maps. Returns (nc, in_maps)."""
    h = np.asarray(inputs["h"], np.float32)
    edge_index = np.asarray(inputs["edge_index"])
    edge_weight = np.asarray(inputs["edge_weight"], np.float32)

    per_core, hpad, gpb, bpc, nsg, npad = _host_prep(
        h, edge_index, edge_weight, n, cores)
    consts = _weights_host(inputs)
    consts["H5"] = hpad

    nc = _build_nc(gpb, bpc, nsg, npad, cores, f32r=f32r)

    in_maps = []
    for c in range(cores):
        m = dict(consts)
        m.update(per_core[c])
        in_maps.append({k: np.ascontiguousarray(v, dtype=v.dtype)
                        for k, v in m.items()})
    return nc, in_maps


def run(inputs, n, cores, f32r=True, run_sim=False, trace=False):
    """Build + execute; returns (out [n,5], BassKernelResults|None)."""
    if run_sim:
        h = np.asarray(inputs["h"], np.float32)
        edge_index = np.asarray(inputs["edge_index"])
        edge_weight = np.asarray(inputs["edge_weight"], np.float32)
        per_core, hpad, gpb, bpc, nsg, npad = _host_prep(
            h, edge_index, edge_weight, n, cores)
        consts = _weights_host(inputs)
        consts["H5"] = hpad
        nc = _build_nc(gpb, bpc, nsg, npad, cores, f32r=f32r)
        in_maps = []
        for c in range(cores):
            m = dict(consts)
            m.update(per_core[c])
            in_maps.append({k: np.ascontiguousarray(v, dtype=v.dtype)
                            for k, v in m.items()})
    else:
        nc, in_maps = prepare(inputs, n, cores, f32r=f32r)

    if run_sim:
        from concourse import bass_interp
        if cores == 1:
            sim = bass_interp.CoreSim(nc)
            sims = [sim]
        else:
            sim = bass_interp.MultiCoreSim(nc, num_cores=cores)
            sims = list(sim.cores.values())
        for ci, cs_ in enumerate(sims):
            for k, v in in_maps[ci].items():
                cs_.tensor(k)[:] = v
        sim.simulate()
        outs = [np.array(cs_.tensor("OUTT")).T for cs_ in sims]
        return np.concatenate(outs, axis=0)[:n].astype(np.float32), None

    from concourse.bass_utils import run_bass_kernel_spmd
    kw = {}
    if trace:
        import os as _os
        _os.makedirs("/tmp/trace0", exist_ok=True)
        kw = dict(trace=True, tmpdir="/tmp/trace0")
    res = run_bass_kernel_spmd(nc, in_maps, core_ids=list(range(cores)), **kw)
    outs = [r["OUTT"].T for r in res.results]
    return np.concatenate(outs, axis=0)[:n].astype(np.float32), res


def kernel(**inputs):
    out, _ = run(inputs, N, 8, f32r=True)
    return out

